# revision 14
# baseline (speedup 1.0000x reference)
"""TRN2 Bass kernel for nn_ContrastiveLoss_45277545235064.

Reference computation (see problem):
    f   = features / ||features||        (row-normalize, fp32)
    E   = exp((f @ f.T) / tau)           [N, N], tau = 0.1
    pos = sum_{same group} E - exp(1/tau)
    neg = sum_{other groups} E
    loss = sum(-log(pos / neg)) / N

Sharding: data-parallel over anchors. Each of the 8 cores computes the
[1024, 8192] slab of E for its anchor rows against the full feature set,
reducing each row on the fly (E never materializes in HBM).

Numerics: the main sweep runs in bf16 (full PE speed). The same-group
positives sit in the 128x128 block-diagonal tiles and include the self term
exp(1/tau) ~ 22026 which is subtracted; a bf16 self term would wreck pos (a
0.4% error there is ~880 absolute vs pos ~ 10). So each core recomputes its
8 diagonal 128x128 blocks in fp32 and splices them in:
    R      = bf16 row sum over all 8192 cols (includes bf16 diag block)
    B16    = bf16 diag-block row sum   (bit-identical recompute)
    B32    = fp32 diag-block row sum
    POS    = fp32 masked (same-group, no self) diag-block row sum
    EII    = fp32 self term
    pos    = POS                       (self excluded => exact cancellation)
    neg    = R - B16 + B32 - POS - EII
Host finishes with loss = mean(log(neg) - log(pos)) in float64.

rsqrt is computed with 6 Newton iterations on the vector engine from a fixed
seed (feature rows are ~N(0,1), so ||x||^2 ~ 256); this avoids the scalar
engine's low-precision Sqrt table and any activation-table switch (Exp is
the only ACT function used).
"""

import sys

sys.path.insert(0, "/opt/trn_rl_repo")

import numpy as np

import concourse.bass as bass  # noqa: F401  (import keeps bass registered)
import concourse.mybir as mybir
import concourse.tile as tile
from concourse import bacc
from concourse.bass_utils import run_bass_kernel_spmd

P = 128
N = 8192
D = 256
CORES = 8
SLAB = N // CORES  # 1024 anchor rows per core
TS = SLAB // P  # 8 anchor tiles per core
CT = N // P  # 64 column tiles
NCH = 512  # matmul moving free dim (one PSUM bank)
NCHUNKS = N // NCH  # 16
GROUP = 8
INV_TAU = 10.0

f32 = mybir.dt.float32
bf16 = mybir.dt.bfloat16
OP = mybir.AluOpType
EXP = mybir.ActivationFunctionType.Exp

_cache: dict = {}


def _build(debug: bool = False, stop_after: int = 6):
    nc = bacc.Bacc(
        "TRN2",
        target_bir_lowering=False,
        debug=debug,
        num_devices=CORES,
    )

    feats_d = nc.dram_tensor("feats", [N, D], f32, kind="ExternalInput")
    slab_d = nc.dram_tensor("slab", [SLAB, D], f32, kind="ExternalInput")
    posmask_d = nc.dram_tensor("posmask", [P, P], f32, kind="ExternalInput")
    eye_d = nc.dram_tensor("eyemask", [P, P], f32, kind="ExternalInput")
    pos_d = nc.dram_tensor("pos", [SLAB], f32, kind="ExternalOutput")
    neg_d = nc.dram_tensor("neg", [SLAB], f32, kind="ExternalOutput")

    W = TS + CT  # 72 row tiles to normalize (slab first, then full)

    with tile.TileContext(nc) as tc:
        with (
            tc.tile_pool(name="persist", bufs=1) as pp,
            tc.tile_pool(name="work", bufs=3) as wp,
            tc.tile_pool(name="psum", bufs=2, space="PSUM") as pq,
        ):
            BIG = 2048  # one PSUM allocation: 4 banks; pool holds 2 = all 8

            # persistent SBUF tensors
            feats_sb = pp.tile([P, CT, D], f32)
            slab_sb = pp.tile([P, TS, D], f32)
            fnb16 = pp.tile([P, W, D], bf16)   # normalized rows, bf16
            fn32 = pp.tile([P, TS, D], f32)    # normalized slab rows, fp32
            fTb_0 = pp.tile([P, N], bf16)
            fTb_1 = pp.tile([P, N], bf16)
            fTs16_0 = pp.tile([P, SLAB], bf16)
            fTs16_1 = pp.tile([P, SLAB], bf16)
            fTs32_0 = pp.tile([P, SLAB], f32)
            fTs32_1 = pp.tile([P, SLAB], f32)
            posmask = pp.tile([P, P], f32)
            eye = pp.tile([P, P], f32)
            eye16 = pp.tile([P, P], bf16)
            ssq = pp.tile([P, W], f32)
            rr = pp.tile([P, W], f32)
            nrt = pp.tile([P, W], f32)
            accs = pp.tile([P, TS * 4], f32)
            B32 = pp.tile([P, TS], f32)
            B16 = pp.tile([P, TS], f32)
            POS = pp.tile([P, TS], f32)
            EII = pp.tile([P, TS], f32)
            Racc = pp.tile([P, TS], f32)
            NEG = pp.tile([P, TS], f32)

            nc.sync.dma_start(posmask[:], posmask_d[:])
            nc.sync.dma_start(eye[:], eye_d[:])
            nc.vector.tensor_copy(eye16[:], eye[:])

            # ---- phase 1: load rows (4 DMA queues) + one-op sum-of-squares
            dma_engines = [nc.sync, nc.scalar]

            def load_and_ssq(src_ap, dst_slice, col):
                dma_engines[col % 2].dma_start(dst_slice, src_ap)
                junk = wp.tile([P, D], f32, tag="ssq_junk")
                nc.vector.scalar_tensor_tensor(
                    junk[:], dst_slice, 1.0, dst_slice, OP.mult, OP.mult,
                    accum_out=ssq[:, col : col + 1],
                )

            for j in range(TS):
                load_and_ssq(slab_d[j * P : (j + 1) * P, :], slab_sb[:, j, :], j)
            for t in range(CT):
                load_and_ssq(feats_d[t * P : (t + 1) * P, :], feats_sb[:, t, :], TS + t)

            # ---- phase 2: rsqrt via 5 Newton iterations (DVE), 2 batches
            nc.vector.memset(rr[:], 0.0625)
            for b0, b1 in [(0, 36), (36, W)]:
                for _ in range(5):
                    nc.vector.tensor_mul(nrt[:, b0:b1], rr[:, b0:b1], rr[:, b0:b1])
                    nc.vector.tensor_mul(nrt[:, b0:b1], nrt[:, b0:b1], ssq[:, b0:b1])
                    nc.vector.tensor_scalar(
                        nrt[:, b0:b1], nrt[:, b0:b1], -0.5, 1.5, OP.mult, OP.add
                    )
                    nc.vector.tensor_mul(rr[:, b0:b1], rr[:, b0:b1], nrt[:, b0:b1])

            # ---- phase 3: normalize + cast on GpSimd (idle engine)
            def rawtile(u):
                return slab_sb[:, u, :] if u < TS else feats_sb[:, u - TS, :]

            for u in range(W):
                nc.gpsimd.tensor_scalar_mul(fnb16[:, u, :], rawtile(u), rr[:, u : u + 1])
            for j in range(TS):
                nc.gpsimd.tensor_scalar_mul(fn32[:, j, :], slab_sb[:, j, :], rr[:, j : j + 1])

            # ---- phase 4: transposes on PE (bf16 full speed), batched psum drains
            # full features -> fTb_k
            for k in range(2):
                ksl = slice(k * P, (k + 1) * P)
                dstb = (fTb_0, fTb_1)[k]
                for g in range(4):
                    big = pq.tile([P, BIG], f32, tag="big")
                    for i in range(16):
                        u = TS + g * 16 + i
                        nc.tensor.matmul(
                            big[:, i * P : (i + 1) * P], fnb16[:, u, ksl], eye16[:],
                            start=True, stop=True,
                        )
                    nc.vector.tensor_copy(dstb[:, g * BIG : (g + 1) * BIG], big[:])
            # slab -> fTs16_k (bf16) and fTs32_k (fp32)
            for k in range(2):
                ksl = slice(k * P, (k + 1) * P)
                big = pq.tile([P, BIG], f32, tag="big")
                for j in range(TS):
                    nc.tensor.matmul(
                        big[:, j * P : (j + 1) * P], fnb16[:, j, ksl], eye16[:],
                        start=True, stop=True,
                    )
                nc.vector.tensor_copy((fTs16_0, fTs16_1)[k][:], big[:, :SLAB])
                big2 = pq.tile([P, BIG], f32, tag="big")
                for j in range(TS):
                    nc.tensor.matmul(
                        big2[:, j * P : (j + 1) * P], fn32[:, j, ksl], eye[:],
                        start=True, stop=True,
                    )
                nc.vector.tensor_copy((fTs32_0, fTs32_1)[k][:], big2[:, :SLAB])

            # ---- phase 5: main bf16 sweep, 2048-wide fused exp + row-sum
            for m in range(TS):
                msl = slice(m * P, (m + 1) * P)
                for ng in range(4):
                    big = pq.tile([P, BIG], f32, tag="big")
                    for k in range(2):
                        fs = (fTs16_0, fTs16_1)[k]
                        fb = (fTb_0, fTb_1)[k]
                        for c in range(4):
                            n = ng * 4 + c
                            nc.tensor.matmul(
                                big[:, c * NCH : (c + 1) * NCH],
                                fs[:, msl],
                                fb[:, n * NCH : (n + 1) * NCH],
                                start=(k == 0),
                                stop=(k == 1),
                            )
                    eo = wp.tile([P, BIG], bf16, tag="eo")
                    col = m * 4 + ng
                    nc.scalar.activation(
                        eo[:], big[:], EXP, scale=INV_TAU,
                        accum_out=accs[:, col : col + 1],
                    )

            # ---- phase 6: diagonal blocks, 4 per activation; sums on DVE
            for h in range(2):
                big32 = pq.tile([P, BIG], f32, tag="big")
                big16 = pq.tile([P, BIG], f32, tag="big")
                for i in range(4):
                    m = h * 4 + i
                    msl = slice(m * P, (m + 1) * P)
                    isl = slice(i * P, (i + 1) * P)
                    for k in range(2):
                        f32k = (fTs32_0, fTs32_1)[k]
                        f16k = (fTs16_0, fTs16_1)[k]
                        nc.tensor.matmul(
                            big32[:, isl], f32k[:, msl], f32k[:, msl],
                            start=(k == 0), stop=(k == 1),
                        )
                        nc.tensor.matmul(
                            big16[:, isl], f16k[:, msl], f16k[:, msl],
                            start=(k == 0), stop=(k == 1),
                        )
                E32h = wp.tile([P, NCH], f32, tag="E32h")
                nc.scalar.activation(E32h[:], big32[:, :NCH], EXP, scale=INV_TAU)
                E16h = wp.tile([P, NCH], f32, tag="E16h")
                nc.scalar.activation(E16h[:], big16[:, :NCH], EXP, scale=INV_TAU)
                for i in range(4):
                    m = h * 4 + i
                    isl = slice(i * P, (i + 1) * P)
                    nc.vector.tensor_reduce(
                        out=B32[:, m : m + 1], in_=E32h[:, isl],
                        axis=mybir.AxisListType.X, op=OP.add,
                    )
                    nc.vector.tensor_reduce(
                        out=B16[:, m : m + 1], in_=E16h[:, isl],
                        axis=mybir.AxisListType.X, op=OP.add,
                    )
                    junk1 = wp.tile([P, P], f32, tag="mjunk1")
                    nc.vector.tensor_mul(junk1[:], E32h[:, isl], posmask[:])
                    nc.vector.tensor_reduce(
                        out=POS[:, m : m + 1], in_=junk1[:],
                        axis=mybir.AxisListType.X, op=OP.add,
                    )
                    junk2 = wp.tile([P, P], f32, tag="mjunk2")
                    nc.vector.tensor_mul(junk2[:], E32h[:, isl], eye[:])
                    nc.vector.tensor_reduce(
                        out=EII[:, m : m + 1], in_=junk2[:],
                        axis=mybir.AxisListType.X, op=OP.add,
                    )

            # ---- phase 7: combine and store
            for m in range(TS):
                nc.vector.tensor_reduce(
                    out=Racc[:, m : m + 1], in_=accs[:, m * 4 : (m + 1) * 4],
                    axis=mybir.AxisListType.X, op=OP.add,
                )
            nc.vector.tensor_sub(NEG[:], Racc[:], B16[:])
            nc.vector.tensor_add(NEG[:], NEG[:], B32[:])
            nc.vector.tensor_sub(NEG[:], NEG[:], POS[:])
            nc.vector.tensor_sub(NEG[:], NEG[:], EII[:])

            nc.sync.dma_start(pos_d.ap().rearrange("(m p) -> p m", p=P), POS[:])
            nc.sync.dma_start(neg_d.ap().rearrange("(m p) -> p m", p=P), NEG[:])

    nc.compile()
    return nc


def _masks_from_num_crops(num_crops: np.ndarray):
    nca = np.asarray(num_crops).astype(np.int64)
    assert int(nca.sum()) == N, f"num_crops sums to {nca.sum()}, expected {N}"
    assert np.all(nca == GROUP), "kernel specialized for constant group size 8"
    blk = np.ones((GROUP, GROUP), dtype=np.float32)
    full = np.kron(np.eye(P // GROUP, dtype=np.float32), blk)
    eye = np.eye(P, dtype=np.float32)
    posmask = full - eye
    return posmask, eye


def _get_program():
    if "nc" not in _cache:
        _cache["nc"] = _build(debug=False)
    return _cache["nc"]


def _run(features: np.ndarray, num_crops: np.ndarray, **spmd_kwargs):
    feats = np.ascontiguousarray(np.asarray(features, dtype=np.float32))
    assert feats.shape == (N, D)
    posmask, eye = _masks_from_num_crops(num_crops)

    nc = _get_program()
    in_maps = [
        {
            "feats": feats,
            "slab": np.ascontiguousarray(feats[c * SLAB : (c + 1) * SLAB]),
            "posmask": posmask,
            "eyemask": eye,
        }
        for c in range(CORES)
    ]
    br = run_bass_kernel_spmd(nc, in_maps, list(range(CORES)), **spmd_kwargs)
    res = br.results
    pos = np.concatenate([res[c]["pos"] for c in range(CORES)]).astype(np.float64)
    neg = np.concatenate([res[c]["neg"] for c in range(CORES)]).astype(np.float64)
    loss = np.mean(np.log(neg) - np.log(pos))
    return np.asarray(loss, dtype=np.float32), br


def kernel(features: np.ndarray, num_crops: np.ndarray) -> np.ndarray:
    loss, _ = _run(features, num_crops)
    return loss
